# revision 10
# baseline (speedup 1.0000x reference)
"""Trainium2 Bass kernel for nn_Decoder (Bahdanau-attention LSTM decoder).

B=256,T=128,ENC=DEC=256,OUT=3. Data-parallel over batch: 8 cores x 32 batch.

Per-core design (feature-major attention pipeline, batch-major LSTM):
  - z2 = W2 @ X^T precomputed once into SBUF, bf16, free order (t,b) "t-major"
  - batch rows live scattered across psum partitions: b -> 32*(b//8) + b%8,
    so score/ctx matmuls split across the PE's 4 column-strips (and 4 row-
    strips for ctx) and run concurrently on the 32x32 subarrays.
  - per step: z1 (PE, f-major) -> b-half-pipelined: broadcast-add over t
    (DVE 2x bf16) -> tanh (ACT) -> scores via diag-masked w3 lhsT into
    scattered psum rows [128,128] -> exp+rowsum (ACT fused accum) ->
    E^T (PE transpose 128x128) -> ARENA copy -> ctx via 128 tiled MMs
    (16 subarrays) -> scale by 1/D -> transposes -> gates MM (fp32r,
    gate order (i,f,o,g) for fused sigmoid) -> LSTM elementwise ->
    state transposes.
  - total_hidden stored f-major in SBUF; head (fc2@fc1 composed on host) is
    one fp32r matmul sweep at the end.
"""

import sys
import numpy as np

sys.path.insert(0, "/opt/trn_rl_repo")

import ml_dtypes

BF16 = ml_dtypes.bfloat16

NCORES = 8
BL = 32          # batch per core
T = 128          # encoder positions == decoder steps
ENC = 256
DEC = 256
OUT = 3
BT = BL * T      # 4096
S = 128          # decoder steps
HB = 16          # batch half

_BUILT = None


def _build_nc():
    from contextlib import ExitStack
    from concourse import bacc, mybir, tile

    dt = mybir.dt
    F32, B16, F32R = dt.float32, dt.bfloat16, dt.float32r
    AF = mybir.ActivationFunctionType

    nc = bacc.Bacc("TRN2", target_bir_lowering=False, debug=False,
                   enable_asserts=False, num_devices=NCORES)

    # ---- DRAM I/O ----
    di = lambda n, sh, d: nc.dram_tensor(n, sh, d, kind="ExternalInput").ap()
    xt = di("xt", [ENC, BT], B16)        # X^T, cols t-major: [e, t*32+b]
    x = di("x", [BT, ENC], B16)          # X, rows b-major: [b*128+t, e]
    y = di("y", [OUT, S * BL], F32R)      # [j, s*32+b]
    w2t = di("w2t", [ENC, ENC], B16)     # attn2_w.T [e, f]
    w1t = di("w1t", [2 * DEC, ENC], B16)  # attn1_w.T [k_hc, f]
    w3d = di("w3d", [128, 2048], B16)    # diag-masked w3 [f_row, fc*1024+b*32+(b%8)]
    bc = di("bc", [ENC, 1], F32)         # attn1_b + attn2_b
    wcy = di("wcy", [OUT, 4 * DEC], F32R)     # W_comb.T rows 0:3 (gate order i,f,o,g)
    wcc = di("wcc", [ENC, 4 * DEC], F32R)     # W_comb.T rows 3:259
    whht = di("whht", [DEC, 4 * DEC], F32R)   # w_hh.T
    gb = di("gb", [1, 4 * DEC], F32R)
    fct = di("fct", [DEC + ENC, OUT], F32R)   # (fc2@fc1).T
    fcb = di("fcb", [1, OUT], F32R)
    onesr = di("onesr", [1, 512], F32R)
    i32 = di("i32", [32, 32], F32)           # identity for transposes
    i128 = di("i128", [128, 128], F32)       # identity for 128-wide transposes
    o = nc.dram_tensor("o", [OUT, S * BL], dt.float32, kind="ExternalOutput").ap()

    with tile.TileContext(nc) as tc, ExitStack() as ctx:
        # ---------------- persistent SBUF ----------------
        P = ctx.enter_context(tc.tile_pool(name="persist", bufs=1))
        Z2 = [P.tile([128, BT], B16, tag=f"z2{i}", name=f"Z2_{i}") for i in range(2)]
        XS = P.tile([128, BL * ENC], B16, tag="xs")          # [t, b*256+e]
        YS = P.tile([OUT, S * BL], F32R, tag="ys")
        W1TS = P.tile([128, 4 * ENC], B16, tag="w1ts")       # [kc*256+f]
        W3DS = P.tile([128, 2048], B16, tag="w3ds")
        BCS = P.tile([128, 2], F32, tag="bcs")
        WCYS = P.tile([OUT, 4 * DEC], F32R, tag="wcys")
        WCCS = P.tile([128, 2 * 4 * DEC], F32R, tag="wccs")
        WHHTS = P.tile([128, 2 * 4 * DEC], F32R, tag="whhts")
        GBS = P.tile([1, 4 * DEC], F32R, tag="gbs")
        FCTS = P.tile([128, 4 * OUT], F32R, tag="fcts")
        FCBS = P.tile([1, OUT], F32R, tag="fcbs")
        ONES = P.tile([1, 512], F32R, tag="ones")
        I32F = P.tile([32, 32], F32, tag="i32f")
        I128F = P.tile([128, 128], F32, tag="i128f")
        TH = [P.tile([128, S * BL], F32R, tag=f"th{i}", name=f"TH_{i}") for i in range(4)]
        ARENA = P.tile([128, 32 * 32], B16, tag="arena")     # ctx lhsT arena, one col per b
        HCT0 = P.tile([128, 128], B16, tag="hct0")           # zero h,c^T step0
        Z128 = P.tile([128, 64], F32R, tag="z128")            # zero h^T fp32 step0
        C0 = P.tile([BL, DEC], F32, tag="c0")
        HALF = P.tile([BL, 1], F32, tag="half")

        # load weights / inputs
        for b in range(BL):
            nc.sync.dma_start(XS[:, b * ENC:(b + 1) * ENC], x[b * T:(b + 1) * T, :])
        nc.sync.dma_start(YS[:], y[:])
        for kc in range(4):
            nc.sync.dma_start(W1TS[:, kc * ENC:(kc + 1) * ENC],
                              w1t[kc * 128:(kc + 1) * 128, :])
        nc.sync.dma_start(W3DS[:], w3d[:])
        for c in range(2):
            nc.sync.dma_start(BCS[:, c:c + 1], bc[c * 128:(c + 1) * 128, :])
        nc.sync.dma_start(WCYS[:], wcy[:])
        for j in range(2):
            nc.sync.dma_start(WCCS[:, j * 1024:(j + 1) * 1024],
                              wcc[j * 128:(j + 1) * 128, :])
            nc.sync.dma_start(WHHTS[:, j * 1024:(j + 1) * 1024],
                              whht[j * 128:(j + 1) * 128, :])
        nc.sync.dma_start(GBS[:], gb[:])
        for kc in range(4):
            nc.sync.dma_start(FCTS[:, kc * OUT:(kc + 1) * OUT],
                              fct[kc * 128:(kc + 1) * 128, :])
        nc.sync.dma_start(FCBS[:], fcb[:])
        nc.sync.dma_start(ONES[:], onesr[:])
        nc.sync.dma_start(I32F[:], i32[:])
        nc.sync.dma_start(I128F[:], i128[:])

        nc.vector.memset(ARENA[:], 0.0)
        nc.vector.memset(HCT0[:], 0.0)
        nc.vector.memset(Z128[:].bitcast(F32), 0.0)
        nc.vector.memset(C0[:], 0.0)
        nc.vector.memset(HALF[:], 0.5)

        # ---------------- z2 precompute ----------------
        with tc.tile_pool(name="xts", bufs=1) as xtp, \
             tc.tile_pool(name="z2ps", bufs=2, space="PSUM") as z2ps, \
             tc.tile_pool(name="w2p", bufs=1) as w2p:
            W2TS = w2p.tile([128, 2 * ENC], B16)
            for ec in range(2):
                nc.sync.dma_start(W2TS[:, ec * ENC:(ec + 1) * ENC],
                                  w2t[ec * 128:(ec + 1) * 128, :])
            XTS = [xtp.tile([128, BT], B16, tag=f"xt{e}", name=f"XTS_{e}") for e in range(2)]
            for ec in range(2):
                nc.sync.dma_start(XTS[ec][:], xt[ec * 128:(ec + 1) * 128, :])
            for fc in range(2):
                for nq in range(8):
                    zp = z2ps.tile([128, 512], F32, tag="zp")
                    for ec in range(2):
                        nc.tensor.matmul(
                            zp[:], W2TS[:, ec * ENC + fc * 128: ec * ENC + fc * 128 + 128],
                            XTS[ec][:, nq * 512:(nq + 1) * 512],
                            start=(ec == 0), stop=(ec == 1))
                    nc.vector.tensor_copy(Z2[fc][:, nq * 512:(nq + 1) * 512], zp[:])

        # ---------------- step pools ----------------
        loop_ctx = ExitStack()
        tin_p = loop_ctx.enter_context(tc.tile_pool(name="tin", bufs=2))
        tout_p = loop_ctx.enter_context(tc.tile_pool(name="tout", bufs=4))
        sb_p = loop_ctx.enter_context(tc.tile_pool(name="small", bufs=2))
        st_p = loop_ctx.enter_context(tc.tile_pool(name="state", bufs=2))
        sc_ps = loop_ctx.enter_context(tc.tile_pool(name="scps", bufs=1, space="PSUM"))
        cx_ps = loop_ctx.enter_context(tc.tile_pool(name="cxps", bufs=1, space="PSUM"))
        g_ps = loop_ctx.enter_context(tc.tile_pool(name="gps", bufs=1, space="PSUM"))
        z1_ps = loop_ctx.enter_context(tc.tile_pool(name="z1ps", bufs=1, space="PSUM"))
        tp_ps = loop_ctx.enter_context(tc.tile_pool(name="tpps", bufs=1, space="PSUM"))

        hct_prev = HCT0          # [128,128] bf16: h^T(2) ++ c^T(2) blocks of 32 cols
        c_prev = C0

        for s in range(S):
            last = (s == S - 1)
            # ---- z1 = W1 @ hc + (b1+b2), f-major [f, b] ----
            z1p = z1_ps.tile([128, 64], F32, tag="z1")
            for fc in range(2):
                for kc in range(4):
                    nc.tensor.matmul(
                        z1p[:, fc * 32:(fc + 1) * 32],
                        W1TS[:, kc * ENC + fc * 128: kc * ENC + fc * 128 + 128],
                        hct_prev[:, kc * 32:(kc + 1) * 32],
                        start=(kc == 0), stop=(kc == 3))
            z1s = sb_p.tile([128, 64], B16, tag="z1s")
            for fc in range(2):
                nc.scalar.activation(z1s[:, fc * 32:(fc + 1) * 32],
                                     z1p[:, fc * 32:(fc + 1) * 32],
                                     AF.Identity, bias=BCS[:, fc:fc + 1])

            # ---- b-half pipelined: add -> tanh -> scores MMs ----
            scp = sc_ps.tile([32, 128], F32, tag="sc")
            for h in range(2):
                touts = []
                for fc in range(2):
                    tin = tin_p.tile([128, T * HB], B16, tag="tin")
                    tin3 = tin[:].rearrange("p (t b) -> p t b", b=HB)
                    z23 = Z2[fc][:].rearrange("p (t b) -> p t b", b=32)[
                        :, :, h * HB:(h + 1) * HB]
                    z1b = z1s[:, None, fc * 32 + h * HB: fc * 32 + (h + 1) * HB
                              ].broadcast_to([128, T, HB])
                    nc.vector.tensor_add(tin3, z23, z1b)
                    tout = tout_p.tile([128, T * HB], B16, tag="tout")
                    nc.scalar.activation(tout[:], tin[:], AF.Tanh)
                    touts.append(tout)
                # 32 MMs this half accumulate into compact scp [32, 128]
                for fc in range(2):
                    t3 = touts[fc][:].rearrange("p (t b) -> p t b", b=HB)
                    for bl in range(HB):
                        b = h * HB + bl
                        nc.tensor.matmul(
                            scp[:],
                            W3DS[:, fc * 1024 + b * 32: fc * 1024 + b * 32 + 32],
                            t3[:, :, bl],
                            start=(h == 0 and fc == 0 and bl == 0),
                            stop=(h == 1 and fc == 1 and bl == HB - 1))

            # ---- softmax pieces ----
            E = sb_p.tile([32, 128], F32, tag="E")
            D = sb_p.tile([32, 1], F32, tag="D")
            nc.scalar.activation(E[:], scp[:], AF.Exp, accum_out=D[:])
            Dinv = sb_p.tile([32, 1], F32, tag="Dinv")
            nc.vector.reciprocal(Dinv[:], D[:])

            # E^T via PE transpose, diag-write into ctx lhsT arena (stride 33)
            etp = tp_ps.tile([128, 128], F32, tag="etp")
            nc.tensor.transpose(etp[:, 0:32], E[:], I32F[:])
            nc.vector.tensor_copy(ARENA[:, 0:32 * 32:33], etp[:, 0:32])

            # ---- context: 32 accumulating MMs -> psum [32,256] b-major ----
            cxp = cx_ps.tile([32, ENC], F32, tag="cx")
            for b in range(BL):
                nc.tensor.matmul(
                    cxp[:], ARENA[:, b * 32:(b + 1) * 32],
                    XS[:, b * ENC:(b + 1) * ENC],
                    start=(b == 0), stop=(b == BL - 1))
            ctxb = sb_p.tile([BL, ENC], F32, tag="ctxb")
            nc.vector.tensor_scalar_mul(ctxb[:], cxp[:], Dinv[:])

            # ctx^T into TH (f-major)
            arena2 = tp_ps.tile([128, 256], F32, tag="arena2")
            for j in range(2):
                nc.tensor.transpose(arena2[:, j * 32: 32 + j * 32],
                                    ctxb[:, j * 128:(j + 1) * 128], I32F[:])
                nc.vector.tensor_copy(TH[2 + j][:, s * 32:(s + 1) * 32],
                                      arena2[:, j * 32: 32 + j * 32])

            if last:
                # h2_127 == h_127: copy previous th h-slots
                for j in range(2):
                    nc.vector.tensor_copy(TH[j][:, s * 32:(s + 1) * 32],
                                          TH[j][:, (s - 1) * 32: s * 32])
                break

            # ---- gates: psum [32, 1024], fp32r streams; order (i,f,o,g) ----
            gp = g_ps.tile([BL, 4 * DEC], F32, tag="g")
            for nh in range(2):
                c0, c1 = nh * 512, nh * 512 + 512
                nc.tensor.matmul(gp[:, c0:c1], ONES[:, 0:32],
                                 GBS[:, c0:c1], start=True, stop=False)
                nc.tensor.matmul(gp[:, c0:c1],
                                 YS[:, s * 32:(s + 1) * 32],
                                 WCYS[:, c0:c1], start=False, stop=False)
                for j in range(2):
                    nc.tensor.matmul(gp[:, c0:c1],
                                     TH[2 + j][:, s * 32:(s + 1) * 32],
                                     WCCS[:, j * 1024 + c0: j * 1024 + c1],
                                     start=False, stop=False)
                for j in range(2):
                    hTj = (Z128[:, j * 32:(j + 1) * 32] if s == 0
                           else TH[j][:, (s - 1) * 32: s * 32])
                    nc.tensor.matmul(gp[:, c0:c1], hTj,
                                     WHHTS[:, j * 1024 + c0: j * 1024 + c1],
                                     start=False, stop=(j == 1))

            # ---- LSTM elementwise (b-major [32, .]), gates (i,f,o,g) ----
            # sigmoid via 0.5*(1+tanh(x/2)): keeps ACT on the exp/tanh
            # table set -> no per-step ACT_TABLE_LOAD pair (~2.6us/step)
            sift = st_p.tile([BL, 768], F32, tag="sift")
            nc.scalar.activation(sift[:], gp[:, 0:768], AF.Tanh, scale=0.5)
            sif = st_p.tile([BL, 768], F32, tag="sif")
            nc.scalar.activation(sif[:], sift[:], AF.Identity, scale=0.5,
                                 bias=HALF[:, 0:1])
            tg = st_p.tile([BL, DEC], F32, tag="tg")
            nc.scalar.activation(tg[:], gp[:, 768:1024], AF.Tanh)
            t1 = st_p.tile([BL, DEC], F32, tag="t1")
            nc.vector.tensor_mul(t1[:], sif[:, 256:512], c_prev[:])
            t2 = st_p.tile([BL, DEC], F32, tag="t2")
            nc.vector.tensor_mul(t2[:], sif[:, 0:256], tg[:])
            cn = st_p.tile([BL, DEC], F32, tag="cn")
            nc.vector.tensor_add(cn[:], t1[:], t2[:])
            tc_ = st_p.tile([BL, DEC], F32, tag="tc")
            nc.scalar.activation(tc_[:], cn[:], AF.Tanh)
            hn = st_p.tile([BL, DEC], F32, tag="hn")
            nc.vector.tensor_mul(hn[:], sif[:, 512:768], tc_[:])

            # ---- state transposes -> TH h-slots (fp32) + HCT bf16 ----
            arena3 = tp_ps.tile([128, 256], F32, tag="arena3")
            hct = sb_p.tile([128, 128], B16, tag="hct")
            for j in range(2):
                nc.tensor.transpose(arena3[:, j * 32: 32 + j * 32],
                                    hn[:, j * 128:(j + 1) * 128], I32F[:])
                nc.vector.tensor_copy(TH[j][:, s * 32:(s + 1) * 32],
                                      arena3[:, j * 32: 32 + j * 32])
                nc.vector.tensor_copy(hct[:, j * 32:(j + 1) * 32],
                                      arena3[:, j * 32: 32 + j * 32])
            for j in range(2):
                nc.tensor.transpose(arena3[:, 64 + j * 32: 96 + j * 32],
                                    cn[:, j * 128:(j + 1) * 128], I32F[:])
                nc.vector.tensor_copy(hct[:, 64 + j * 32: 96 + j * 32],
                                      arena3[:, 64 + j * 32: 96 + j * 32])

            hct_prev = hct
            c_prev = cn

        loop_ctx.close()

        # ---------------- output head ----------------
        with tc.tile_pool(name="ops", bufs=2, space="PSUM") as ops, \
             tc.tile_pool(name="ost", bufs=2) as ost:
            for nq in range(8):
                op = ops.tile([OUT, 512], F32, tag="op")
                for kc in range(4):
                    nc.tensor.matmul(op[:], FCTS[:, kc * OUT:(kc + 1) * OUT],
                                     TH[kc][:, nq * 512:(nq + 1) * 512],
                                     start=(kc == 0), stop=False)
                nc.tensor.matmul(op[:], FCBS[:], ONES[:],
                                 start=False, stop=True)
                ot = ost.tile([OUT, 512], F32, tag="ot")
                nc.vector.tensor_copy(ot[:], op[:])
                nc.sync.dma_start(o[:, nq * 512:(nq + 1) * 512], ot[:])

    nc.compile()
    return nc


def _host_prep(inputs):
    f32 = np.float32
    ie = np.asarray(inputs["input_encoded"], f32)      # [256,128,256]
    ys = np.asarray(inputs["y_seq"], f32)              # [256,128,3]
    a1w = np.asarray(inputs["attn1_w"], f32)           # [256,512]
    a1b = np.asarray(inputs["attn1_b"], f32)
    a2w = np.asarray(inputs["attn2_w"], f32)
    a2b = np.asarray(inputs["attn2_b"], f32)
    a3w = np.asarray(inputs["attn3_w"], f32)           # [1,256]
    tw = np.asarray(inputs["tilde_w"], f32)            # [512,259]
    tb = np.asarray(inputs["tilde_b"], f32)
    wih = np.asarray(inputs["w_ih"], f32)              # [1024,512]
    whh = np.asarray(inputs["w_hh"], f32)              # [1024,256]
    bih = np.asarray(inputs["b_ih"], f32)
    bhh = np.asarray(inputs["b_hh"], f32)
    f1w = np.asarray(inputs["fc1_w"], f32)             # [256,512]
    f1b = np.asarray(inputs["fc1_b"], f32)
    f2w = np.asarray(inputs["fc2_w"], f32)             # [3,256]
    f2b = np.asarray(inputs["fc2_b"], f32)

    wcomb = wih @ tw                                    # [1024,259]
    wcombT = np.ascontiguousarray(wcomb.T)              # [259,1024]
    gbias = wih @ tb + bih + bhh                        # [1024]
    fc = f2w @ f1w                                      # [3,512]
    fcbias = f2w @ f1b + f2b                            # [3]

    # permute gate columns (i,f,g,o) -> (i,f,o,g) so sigmoid spans one slice
    perm = np.concatenate([np.arange(0, 512), np.arange(768, 1024),
                           np.arange(512, 768)])
    wcombT = wcombT[:, perm]
    gbias = gbias[perm]

    w3diag = np.zeros((128, 2, 32, 32), f32)
    for fc_ in range(2):
        w3diag[:, fc_, np.arange(32), np.arange(32)] = \
            a3w[0, fc_ * 128:(fc_ + 1) * 128][:, None]
    w3diag = w3diag.reshape(128, 2048)

    shared = {
        "w2t": np.ascontiguousarray(a2w.T).astype(BF16),
        "w1t": np.ascontiguousarray(a1w.T).astype(BF16),
        "w3d": w3diag.astype(BF16),
        "bc": (a1b + a2b)[:, None].astype(f32),
        "wcy": np.ascontiguousarray(wcombT[0:3]).astype(f32),
        "wcc": np.ascontiguousarray(wcombT[3:259]).astype(f32),
        "whht": np.ascontiguousarray(whh.T[:, perm]).astype(f32),
        "gb": gbias[None, :].astype(f32),
        "fct": np.ascontiguousarray(fc.T).astype(f32),
        "fcb": fcbias[None, :].astype(f32),
        "onesr": np.ones((1, 512), f32),
        "i32": np.eye(32, dtype=f32),
        "i128": np.eye(128, dtype=f32),
    }
    in_maps = []
    for i in range(NCORES):
        b0 = i * BL
        xe = ie[b0:b0 + BL]                            # [32,128,256]
        m = dict(shared)
        m["xt"] = np.ascontiguousarray(
            xe.transpose(2, 1, 0).reshape(ENC, BT)).astype(BF16)
        m["x"] = xe.reshape(BT, ENC).astype(BF16)
        m["y"] = np.ascontiguousarray(
            ys[b0:b0 + BL].transpose(2, 1, 0).reshape(OUT, S * BL)).astype(f32)
        in_maps.append(m)
    return in_maps


def kernel(**inputs):
    global _BUILT
    from concourse import bass_utils
    if _BUILT is None:
        _BUILT = _build_nc()
    nc = _BUILT
    import os
    in_maps = _host_prep(inputs)
    trace = bool(int(os.environ.get("KERNEL_TRACE", "0")))
    res = bass_utils.run_bass_kernel_spmd(nc, in_maps, core_ids=list(range(NCORES)),
                                          trace=trace)
    if trace:
        print(f"HW exec time: {res.exec_time_ns} ns  (mean {res.mean_exec_time_ns})")
        globals()['_LAST_RESULTS'] = res
    outs = []
    for i in range(NCORES):
        oc = res.results[i]["o"]                       # [3, 4096] (j, s*32+b)
        outs.append(oc.reshape(OUT, S, BL).transpose(2, 1, 0))
    return np.concatenate(outs, axis=0).astype(np.float32)


if __name__ == "__main__":
    rng = np.random.default_rng(0)
    pass
